# revision 15
# baseline (speedup 1.0000x reference)
"""TRN2 Bass kernel for nn_Codec (VQ autoencoder), 8-way data-parallel over batch.

Contract: kernel(**inputs) takes the FULL unsharded inputs (as produced by
setup_inputs()) and returns the FULL [4096, 3, 32, 32] float32 output.

Design (per core, batch slice of 512 rows, activations kept transposed so the
batch lives on the free dimension and weight tiles stream straight from DRAM):
  xT [3072,512] --GEMM1 f16--> hT --tanh--> GEMM2 f16 (PSUM-held accum)
  --> h2T [256,512] --tanh--> VQ (PE distance matmuls incl. the |x|^2 row via a
  squared-chunk matmul, batched [128,1024] sqrt/exp/reduce, DVE argmax,
  one-hot matmul gather; jax threefry noise precomputed on host, bit-exact)
  --> yT [256,512] bf16 --decoder GEMM1 bf16--> y2T [16384,512] bf16
  --decoder GEMM2 bf16--> outT [3072,512] --> host gathers + transposes.

Schedule notes (v2): the first matmul's inputs (w1e tile 0 + xT) dispatch
first on the sync DMA queue while constants/noise/biases dispatch on the
Activation hwdge queue; a short warm-up matmul burst lifts the PE out of the
HAM-throttled state before real work lands; VQ avoids the softmax max-shift
(d >= 0 so exp(-d/8) cannot overflow) and scores argmax(e - s*u) which is
monotone-equivalent to argmax(e/s - u); decoder outputs DMA on the Activation
queue so they never queue behind w2d weight loads.
"""
import os
import sys
from contextlib import ExitStack

import numpy as np

for _p in ("/opt/trn_rl_repo", "/root/.axon_site/_ro/trn_rl_repo"):
    if os.path.isdir(_p) and _p not in sys.path:
        sys.path.append(_p)

import concourse.bass as bass  # noqa: E402
import concourse.tile as tile  # noqa: E402
from concourse import bacc, mybir  # noqa: E402
from concourse.bass_utils import run_bass_kernel_spmd  # noqa: E402

F32 = mybir.dt.float32
BF16 = mybir.dt.bfloat16
F16 = mybir.dt.float16
F32R = mybir.dt.float32r
U32 = mybir.dt.uint32
I32 = mybir.dt.int32
AF = mybir.ActivationFunctionType
ALU = mybir.AluOpType
AX = mybir.AxisListType

N_CORES = 8
BTOT = 4096
B = BTOT // N_CORES          # 512 batch rows per core
IMG = 3 * 32 * 32            # 3072
HID = 16384
NCODE = 256
DCODE = 64
BT = B // 128                # 4
KT1 = IMG // 128             # 24
MT1 = HID // 128             # 128
NT2 = IMG // 128             # 24


# ---------------------------------------------------------------------------
# numpy reimplementation of jax threefry2x32 noise (bit-exact, partitionable)
# ---------------------------------------------------------------------------
def _rotl(x, r):
    return ((x << np.uint32(r)) | (x >> np.uint32(32 - r))) & np.uint32(0xFFFFFFFF)


def _threefry_core(key, x0, x1):
    ks0, ks1 = np.uint32(key[0]), np.uint32(key[1])
    ks2 = np.uint32(ks0 ^ ks1 ^ np.uint32(0x1BD11BDA))
    rotations = [(13, 15, 26, 6), (17, 29, 16, 24)]
    x0 = (x0 + ks0).astype(np.uint32)
    x1 = (x1 + ks1).astype(np.uint32)
    ks = [ks1, ks2, ks0, ks1, ks2, ks0]
    for i in range(5):
        for r in rotations[i % 2]:
            x0 = (x0 + x1).astype(np.uint32)
            x1 = _rotl(x1, r)
            x1 = (x0 ^ x1).astype(np.uint32)
        x0 = (x0 + ks[i]).astype(np.uint32)
        x1 = (x1 + ks[i + 1] + np.uint32(i + 1)).astype(np.uint32)
    return x0, x1


def _fold_in(key, data):
    return _threefry_core(key, np.array([0], np.uint32),
                          np.array([data], np.uint32))


def _uniform_f32(key, n):
    b0, b1 = _threefry_core(key, np.zeros(n, np.uint32),
                            np.arange(n, dtype=np.uint32))
    bits = (b0 ^ b1).astype(np.uint32)
    f = ((bits >> np.uint32(9)) | np.uint32(0x3F800000)).view(np.float32)
    return f - np.float32(1.0)


def _noise_tables():
    key = np.array([0, 12345], dtype=np.uint32)  # jax.random.key(12345)
    out = []
    for i in (1, 2, 3, 4):
        k = _fold_in(key, i)
        k = np.array([k[0][0], k[1][0]], np.uint32)
        out.append(_uniform_f32(k, BTOT * NCODE).reshape(BTOT, NCODE))
    return np.stack(out)  # [4, BTOT, 256]


# ---------------------------------------------------------------------------
# kernel builder
# ---------------------------------------------------------------------------
def _build_kernel():
    nc = bacc.Bacc("TRN2", target_bir_lowering=False, debug=False)

    xT_d = nc.dram_tensor("xT", [KT1, 128, B], F16, kind="ExternalInput").ap()
    w1e_d = nc.dram_tensor("w1e", [MT1, 128, IMG], F16, kind="ExternalInput").ap()
    w2e_d = nc.dram_tensor("w2e", [MT1, 128, NCODE], F16, kind="ExternalInput").ap()
    b1e_d = nc.dram_tensor("b1e", [128, MT1], F32, kind="ExternalInput").ap()
    b2e_d = nc.dram_tensor("b2e", [128, 2], F32, kind="ExternalInput").ap()
    cbm2T_d = nc.dram_tensor("cbm2T", [DCODE, NCODE], F32, kind="ExternalInput").ap()
    c2_d = nc.dram_tensor("c2row", [1, NCODE], F32, kind="ExternalInput").ap()
    cbk_d = nc.dram_tensor("cbk", [2, 128, DCODE], F32, kind="ExternalInput").ap()
    u_d = nc.dram_tensor("u", [BT, 128, 4 * NCODE], F32, kind="ExternalInput").ap()
    w1d_d = nc.dram_tensor("w1d", [MT1 // 4, 128, 4 * NCODE], BF16, kind="ExternalInput").ap()
    b1d_d = nc.dram_tensor("b1d", [128, MT1], F32, kind="ExternalInput").ap()
    w2d_d = nc.dram_tensor("w2d", [NT2, 128, HID], BF16, kind="ExternalInput").ap()
    b2d_d = nc.dram_tensor("b2d", [128, NT2], F32, kind="ExternalInput").ap()
    outT_d = nc.dram_tensor("outT", [NT2, 128, B], F32, kind="ExternalOutput").ap()

    with tile.TileContext(nc) as tc, ExitStack() as octx:
        const_pool = octx.enter_context(tc.tile_pool(name="const", bufs=1))
        upool = octx.enter_context(tc.tile_pool(name="u", bufs=1))
        h2T_pool = octx.enter_context(tc.tile_pool(name="h2T", bufs=2))
        yT_pool = octx.enter_context(tc.tile_pool(name="yT", bufs=2))

        h2T_sb = [h2T_pool.tile([128, B], F32, tag=f"h2T{i}", name=f"h2T{i}")
                  for i in range(2)]

        # ---------------- encoder ----------------
        with ExitStack() as ctx:
            xpool = ctx.enter_context(tc.tile_pool(name="x", bufs=1))
            w1pool = ctx.enter_context(tc.tile_pool(name="w1", bufs=4))
            w2pool = ctx.enter_context(tc.tile_pool(name="w2", bufs=3))
            hpool = ctx.enter_context(tc.tile_pool(name="h", bufs=4))
            warmpool = ctx.enter_context(tc.tile_pool(name="wrm", bufs=1))
            gpsum = ctx.enter_context(tc.tile_pool(name="gps", bufs=3, space="PSUM"))
            h2psum = ctx.enter_context(tc.tile_pool(name="h2ps", bufs=1, space="PSUM"))
            wmpsum = ctx.enter_context(tc.tile_pool(name="wmps", bufs=1, space="PSUM"))

            # PE warm-up: a short matmul burst on a memset tile pulls the HAM
            # clock gate to 8/8 while the first real weight/x DMAs land.
            wsrc = warmpool.tile([128, B], F16)
            nc.vector.memset(wsrc[:], 0.0)
            wps = wmpsum.tile([128, B], F32)
            NWARM = 24
            for i in range(NWARM):
                nc.tensor.matmul(wps[:], wsrc[:, 0:128], wsrc[:],
                                 start=(i == 0), stop=(i == NWARM - 1))

            # first real loads: w1e tile 0 on the sync queue, x on the
            # Activation queue so the two DMA streams run in parallel.
            w1_first = w1pool.tile([128, IMG], F16, tag="w1", name="w1_first")
            nc.sync.dma_start(w1_first[:], w1e_d[0])
            x_sb = xpool.tile([128, KT1 * B], F16)
            XC = 6
            for k0 in range(0, KT1, XC):
                nc.scalar.dma_start(
                    x_sb[:, k0 * B:(k0 + XC) * B].rearrange(
                        "p (a c) -> p a c", a=XC),
                    xT_d[k0:k0 + XC].rearrange("a b c -> b a c"))
            xt = [x_sb[:, kt * B:(kt + 1) * B] for kt in range(KT1)]

            # constants (Activation queue; none are needed before ~t+13us)
            b1e_sb = const_pool.tile([128, MT1], F32)
            nc.scalar.dma_start(b1e_sb[:], b1e_d[:])
            b2e_sb = const_pool.tile([128, 2], F32)
            nc.scalar.dma_start(b2e_sb[:], b2e_d[:])
            b1d_sb = const_pool.tile([128, MT1], F32)
            nc.scalar.dma_start(b1d_sb[:], b1d_d[:])
            b2d_sb = const_pool.tile([128, NT2], F32)
            nc.scalar.dma_start(b2d_sb[:], b2d_d[:])
            cbm2T_sb = const_pool.tile([DCODE, NCODE], F32)
            nc.scalar.dma_start(cbm2T_sb[:], cbm2T_d[:])
            c2_sb = const_pool.tile([1, NCODE], F32)
            nc.scalar.dma_start(c2_sb[:], c2_d[:])
            cbk_sb = []
            for pt in range(2):
                t = const_pool.tile([128, DCODE], F32, tag=f"cbk{pt}", name=f"cbk{pt}")
                nc.scalar.dma_start(t[:], cbk_d[pt])
                cbk_sb.append(t)
            u_sb = []
            for bt in range(BT):
                t = upool.tile([128, 4 * NCODE], F32, tag=f"u{bt}", name=f"u{bt}")
                nc.scalar.dma_start(t[:], u_d[bt])
                u_sb.append(t)
            ones_sb = const_pool.tile([1, 128], F32)
            nc.vector.memset(ones_sb[:], 1.0)
            ones_r = const_pool.tile([1, 128], F32)
            nc.vector.memset(ones_r[:], 1.0)
            ones256_sb = const_pool.tile([DCODE, NCODE], F32)
            nc.vector.memset(ones256_sb[:], 1.0)
            iota_i = const_pool.tile([128, 1], I32)
            nc.gpsimd.iota(iota_i[:], [[0, 1]], base=0, channel_multiplier=1)
            iota_col = []
            for pt in range(2):
                t = const_pool.tile([128, 1], F32, tag=f"iotac{pt}", name=f"iotac{pt}")
                if pt == 0:
                    nc.vector.tensor_copy(t[:], iota_i[:])
                else:
                    nc.vector.tensor_scalar_add(t[:], iota_col[0][:], float(128))
                iota_col.append(t)
            iota_row_i = const_pool.tile([128, 128], I32)
            nc.gpsimd.iota(iota_row_i[:], [[1, 128]], base=0, channel_multiplier=0)
            iota_row_f = const_pool.tile([128, 128], F32)
            nc.vector.tensor_copy(iota_row_f[:], iota_row_i[:])
            ident = const_pool.tile([128, 128], F32)
            nc.vector.tensor_scalar(ident[:], iota_row_f[:], iota_col[0][:], None,
                                    op0=ALU.is_equal)

            h2t_ps = [h2psum.tile([128, B], F32, tag=f"h2t{i}", name=f"h2t{i}")
                      for i in range(2)]

            prev = None
            for mt in range(MT1):
                if mt == 0:
                    w1 = w1_first
                else:
                    w1 = w1pool.tile([128, IMG], F16, tag="w1")
                    nc.sync.dma_start(w1[:], w1e_d[mt])
                w2 = w2pool.tile([128, NCODE], F16, tag="w2")
                nc.scalar.dma_start(w2[:], w2e_d[mt])
                p = gpsum.tile([128, B], F32, tag="gp")
                for kt in range(KT1):
                    nc.tensor.matmul(p[:], w1[:, kt * 128:(kt + 1) * 128], xt[kt],
                                     start=(kt == 0), stop=(kt == KT1 - 1))
                ht = hpool.tile([128, B], F16, tag="ht")
                nc.scalar.activation(ht[:], p[:], AF.Tanh, bias=b1e_sb[:, mt:mt + 1])
                if prev is not None:
                    pw2, pht, pmt = prev
                    for i in range(2):
                        nc.tensor.matmul(h2t_ps[i][:], pw2[:, i * 128:(i + 1) * 128],
                                         pht[:], start=(pmt == 0), stop=False)
                prev = (w2, ht, mt)
            pw2, pht, pmt = prev
            for i in range(2):
                nc.tensor.matmul(h2t_ps[i][:], pw2[:, i * 128:(i + 1) * 128], pht[:],
                                 start=(pmt == 0), stop=True)
            for i in range(2):
                nc.scalar.activation(h2T_sb[i][:], h2t_ps[i][:], AF.Tanh,
                                     bias=b2e_sb[:, i:i + 1])

        # ---------------- VQ ----------------
        yT_sb = [yT_pool.tile([128, B], BF16, tag=f"yT{i}", name=f"yT{i}")
                 for i in range(2)]
        with ExitStack() as ctx:
            h2bpool = ctx.enter_context(tc.tile_pool(name="h2b", bufs=1))
            vqpool = ctx.enter_context(tc.tile_pool(name="vq", bufs=1))
            spool = ctx.enter_context(tc.tile_pool(name="small", bufs=1))
            ohpool = ctx.enter_context(tc.tile_pool(name="oh", bufs=3))
            pdps = ctx.enter_context(tc.tile_pool(name="pdps", bufs=2, space="PSUM"))
            pps = ctx.enter_context(tc.tile_pool(name="pps", bufs=1, space="PSUM"))
            ibps = ctx.enter_context(tc.tile_pool(name="ibps", bufs=1, space="PSUM"))
            yps = ctx.enter_context(tc.tile_pool(name="yps", bufs=2, space="PSUM"))

            # f32 chunk slices of h2T (base partition 0) + their squares
            h2c, sqc = [], []
            for c in range(4):
                hc = h2bpool.tile([DCODE, B], F32, tag=f"h2c{c}", name=f"h2c{c}")
                nc.vector.tensor_copy(
                    hc[:], h2T_sb[c // 2][(c % 2) * DCODE:(c % 2 + 1) * DCODE, :])
                h2c.append(hc)
                sq = h2bpool.tile([DCODE, B], F32, tag=f"sq{c}", name=f"sq{c}")
                nc.vector.scalar_tensor_tensor(sq[:], hc[:], 1.0, hc[:],
                                               op0=ALU.mult, op1=ALU.mult)
                sqc.append(sq)

            idxT = {}
            for c in range(4):
                idxT[c] = spool.tile([1, B], F32, tag=f"idxT{c}", name=f"idxT{c}")

            # wave 1: distance matmuls + relu into SBUF per bt
            # (-2 x.c + c^2 + x^2, the last via the squared-chunk ones matmul)
            qts = []
            for bt in range(BT):
                bs = slice(bt * 128, (bt + 1) * 128)
                pd = pdps.tile([128, 4 * NCODE], F32, tag="pd")
                for c in range(4):
                    cs = slice(c * NCODE, (c + 1) * NCODE)
                    nc.tensor.matmul(pd[:, cs], h2c[c][:, bs], cbm2T_sb[:],
                                     start=True, stop=False)
                    nc.tensor.matmul(pd[:, cs], ones_sb[:], c2_sb[:],
                                     start=False, stop=False)
                    nc.tensor.matmul(pd[:, cs], sqc[c][:, bs], ones256_sb[:],
                                     start=False, stop=True)
                qt = vqpool.tile([128, 4 * NCODE], F32, tag=f"qt{bt}",
                                 name=f"qt{bt}")
                nc.vector.tensor_scalar(qt[:], pd[:], 0.0, None, op0=ALU.max)
                qts.append(qt)
            # wave 2: sqrt for all bt (one ACT table load), then exp for all bt
            dts = []
            for bt in range(BT):
                dt_ = vqpool.tile([128, 4 * NCODE], F32, tag=f"dt{bt}",
                                  name=f"dt{bt}")
                nc.scalar.activation(dt_[:], qts[bt][:], AF.Sqrt)
                dts.append(dt_)
            ets = []
            for bt in range(BT):
                et = vqpool.tile([128, 4 * NCODE], F32, tag=f"et{bt}",
                                 name=f"et{bt}")
                nc.scalar.activation(et[:], dts[bt][:], AF.Exp, scale=-0.125)
                ets.append(et)
            # wave 3a: row sums, reciprocals, scores r - u (argmax-equivalent
            # e*(1/s) - u) via a stride-0 broadcast of 1/s over each segment
            tts = []
            for bt in range(BT):
                et = ets[bt]
                s4 = spool.tile([128, 4], F32, tag=f"s4_{bt}", name=f"s4_{bt}")
                nc.vector.tensor_reduce(
                    s4[:], et[:].rearrange("a (b c) -> a b c", b=4),
                    axis=AX.X, op=ALU.add)
                r4 = spool.tile([128, 4], F32, tag=f"r4_{bt}", name=f"r4_{bt}")
                nc.vector.reciprocal(r4[:], s4[:])
                rb = dts[bt]  # dead after exp; reuse as e*(1/s) scratch
                nc.vector.scalar_tensor_tensor(
                    rb[:].rearrange("a (b c) -> a b c", b=4),
                    et[:].rearrange("a (b c) -> a b c", b=4), 1.0,
                    r4[:].unsqueeze(2).broadcast_to([128, 4, NCODE]),
                    op0=ALU.mult, op1=ALU.mult)
                tt = vqpool.tile([128, 4 * NCODE], F32, tag=f"tt{bt}",
                                 name=f"tt{bt}")
                nc.vector.scalar_tensor_tensor(tt[:], rb[:], 1.0, u_sb[bt][:],
                                               op0=ALU.mult, op1=ALU.subtract)
                tts.append(tt)
            # wave 3b: chunk-major argmax + one-hot gather so yT k-tile 0
            # (chunks 0,1) completes early and decoder GEMM1 can begin
            for c in range(4):
                cs = slice(c * NCODE, (c + 1) * NCODE)
                for bt in range(BT):
                    bs = slice(bt * 128, (bt + 1) * 128)
                    mx8 = spool.tile([128, 8], F32, tag="mx8")
                    nc.vector.max(mx8[:], tts[bt][:, cs])
                    idx8 = spool.tile([128, 8], U32, tag="idx8")
                    nc.vector.max_index(idx8[:], mx8[:], tts[bt][:, cs])
                    idxf = spool.tile([128, 1], F32, tag="idxf")
                    nc.vector.tensor_copy(idxf[:], idx8[:, 0:1])
                    pidx = pps.tile([1, 128], F32, tag="pidx")
                    nc.tensor.transpose(pidx[:], idxf[:], ident[:])
                    nc.vector.tensor_copy(idxT[c][0:1, bs], pidx[:])
                ibp = ibps.tile([128, B], F32, tag="ib")
                nc.tensor.matmul(ibp[:], ones_r[:], idxT[c][:],
                                 start=True, stop=True)
                yp = yps.tile([DCODE, B], F32, tag="yp")
                for pt in range(2):
                    oh = ohpool.tile([128, B], F32, tag="oh")
                    nc.vector.tensor_scalar(oh[:], ibp[:], iota_col[pt][:], None,
                                            op0=ALU.is_equal)
                    nc.tensor.matmul(yp[:], cbk_sb[pt][:], oh[:],
                                     start=(pt == 0), stop=(pt == 1))
                nc.vector.tensor_copy(
                    yT_sb[c // 2][(c % 2) * DCODE:(c % 2 + 1) * DCODE, :], yp[:])

        # ---------------- decoder ----------------
        with ExitStack() as ctx:
            w1dpool = ctx.enter_context(tc.tile_pool(name="w1d", bufs=3))
            y2pool = ctx.enter_context(tc.tile_pool(name="y2", bufs=1))
            w2dpool = ctx.enter_context(tc.tile_pool(name="w2d", bufs=3))
            opool = ctx.enter_context(tc.tile_pool(name="osb", bufs=3))
            dps = ctx.enter_context(tc.tile_pool(name="dps", bufs=6, space="PSUM"))
            eps = ctx.enter_context(tc.tile_pool(name="eps", bufs=2, space="PSUM"))

            y2T = y2pool.tile([128, MT1 * B], BF16)
            KQ = 4
            KQL = MT1 // KQ
            po0 = eps.tile([128, B], F32, tag="ep", name="ep0")
            w2d0 = {}

            def nt0_step(kt):
                kq = kt // KQL
                if kt % KQL == 0 and kq not in w2d0:
                    w2d0[kq] = w2dpool.tile([128, KQL * 128], BF16, tag="w2d",
                                            name=f"w2d0_{kq}")
                    nc.sync.dma_start(
                        w2d0[kq][:],
                        w2d_d[0, :, kq * KQL * 128:(kq + 1) * KQL * 128])
                if (kt + 16) % KQL == 0 and kq + 1 < KQ and kq + 1 not in w2d0:
                    w2d0[kq + 1] = w2dpool.tile([128, KQL * 128], BF16, tag="w2d",
                                                name=f"w2d0p_{kq + 1}")
                    nc.sync.dma_start(
                        w2d0[kq + 1][:],
                        w2d_d[0, :, (kq + 1) * KQL * 128:(kq + 2) * KQL * 128])
                kk = kt % KQL
                nc.tensor.matmul(po0[:], w2d0[kq][:, kk * 128:(kk + 1) * 128],
                                 y2T[:, kt * B:(kt + 1) * B],
                                 start=(kt == 0), stop=(kt == MT1 - 1))

            w2d0[0] = w2dpool.tile([128, KQL * 128], BF16, tag="w2d",
                                   name="w2d0_pre")
            nc.sync.dma_start(w2d0[0][:], w2d_d[0, :, 0:KQL * 128])
            for mg in range(MT1 // 4):
                w1d_sb = w1dpool.tile([128, 4 * NCODE], BF16, tag="w1d")
                nc.sync.dma_start(w1d_sb[:], w1d_d[mg])
                for ml in range(4):
                    mt = mg * 4 + ml
                    p = dps.tile([128, B], F32, tag="dp")
                    nc.tensor.matmul(p[:], w1d_sb[:, ml * 256:ml * 256 + 128],
                                     yT_sb[0][:], start=True, stop=False)
                    nc.tensor.matmul(p[:], w1d_sb[:, ml * 256 + 128:ml * 256 + 256],
                                     yT_sb[1][:], start=False, stop=True)
                    nc.scalar.activation(y2T[:, mt * B:(mt + 1) * B], p[:], AF.Tanh,
                                         bias=b1d_sb[:, mt:mt + 1])
                    if mg >= 2:
                        nt0_step((mg - 2) * 4 + ml)
            for kt in range((MT1 // 4 - 2) * 4, MT1):
                nt0_step(kt)
            osb0 = opool.tile([128, B], F32, tag="osb")
            nc.vector.tensor_scalar_add(osb0[:], po0[:], b2d_sb[:, 0:1])
            nc.scalar.dma_start(outT_d[0], osb0[:])

            for nt in range(1, NT2):
                po = eps.tile([128, B], F32, tag="ep")
                for kq in range(KQ):
                    w2d_sb = w2dpool.tile([128, KQL * 128], BF16, tag="w2d")
                    nc.sync.dma_start(
                        w2d_sb[:], w2d_d[nt, :, kq * KQL * 128:(kq + 1) * KQL * 128])
                    for kk in range(KQL):
                        kt = kq * KQL + kk
                        nc.tensor.matmul(po[:], w2d_sb[:, kk * 128:(kk + 1) * 128],
                                         y2T[:, kt * B:(kt + 1) * B],
                                         start=(kt == 0), stop=(kt == MT1 - 1))
                osb = opool.tile([128, B], F32, tag="osb")
                nc.vector.tensor_scalar_add(osb[:], po[:], b2d_sb[:, nt:nt + 1])
                nc.scalar.dma_start(outT_d[nt], osb[:])

    nc.compile()
    return nc


def _to_bf16(x):
    import ml_dtypes
    return np.asarray(x, np.float32).astype(ml_dtypes.bfloat16)


def _prepare_in_maps(x, wb1e, wb2e, wb1d, wb2d, cb, noise_level, noises):
    W1e, b1e = wb1e[:-1], wb1e[-1]
    W2e, b2e = wb2e[:-1], wb2e[-1]
    W1d, b1d = wb1d[:-1], wb1d[-1]
    W2d, b2d = wb2d[:-1], wb2d[-1]

    xT = np.ascontiguousarray(x.T)  # [IMG, BTOT]
    w1e_p = np.ascontiguousarray(
        W1e.reshape(KT1, 128, MT1, 128).transpose(2, 1, 0, 3)).reshape(
            MT1, 128, IMG).astype(np.float16)
    w2e_p = np.ascontiguousarray(W2e.reshape(MT1, 128, NCODE)).astype(np.float16)
    b1e_p = np.ascontiguousarray(b1e.reshape(MT1, 128).T)
    b2e_p = np.ascontiguousarray(b2e.reshape(2, 128).T)
    cbm2T = np.ascontiguousarray((-2.0 * cb.T).astype(np.float32))
    c2row = np.ascontiguousarray((cb * cb).sum(1, dtype=np.float32)[None, :])
    cbk = np.ascontiguousarray(cb.reshape(2, 128, DCODE))
    w1d_p = _to_bf16(np.ascontiguousarray(
        W1d.reshape(2, 128, MT1, 128).transpose(2, 1, 0, 3)).reshape(MT1, 128, NCODE))
    w1d_p = np.ascontiguousarray(
        w1d_p.reshape(MT1 // 4, 4, 128, NCODE).transpose(0, 2, 1, 3)).reshape(
            MT1 // 4, 128, 4 * NCODE)
    b1d_p = np.ascontiguousarray(b1d.reshape(MT1, 128).T)
    w2d_p = _to_bf16(np.ascontiguousarray(
        W2d.reshape(MT1, 128, NT2, 128).transpose(2, 1, 0, 3)).reshape(NT2, 128, HID))
    b2d_p = np.ascontiguousarray(b2d.reshape(NT2, 128).T)
    u_all = (np.float32(noise_level) * noises).astype(np.float32)  # [4, BTOT, 256]

    shared = {
        "w1e": w1e_p, "w2e": w2e_p, "b1e": b1e_p, "b2e": b2e_p,
        "cbm2T": cbm2T, "c2row": c2row, "cbk": cbk,
        "w1d": w1d_p, "b1d": b1d_p, "w2d": w2d_p, "b2d": b2d_p,
    }
    in_maps = []
    for c in range(N_CORES):
        sl = slice(c * B, (c + 1) * B)
        m = dict(shared)
        m["xT"] = np.ascontiguousarray(xT[:, sl]).reshape(KT1, 128, B).astype(np.float16)
        # [4, B, 256] -> [B, 4, 256] -> [BT, 128, 4*256] (bt-major, c inner)
        m["u"] = np.ascontiguousarray(
            u_all[:, sl, :].transpose(1, 0, 2)).reshape(BT, 128, 4 * NCODE)
        in_maps.append(m)
    return in_maps


_CACHE = {}


def kernel(x, wb1_encoder, wb2_encoder, wb1_decoder, wb2_decoder,
           codebook1, codebook2, codebook3, codebook4, noise_level,
           **_unused):
    x2d = np.ascontiguousarray(np.asarray(x, np.float32).reshape(BTOT, IMG))
    if "nc" not in _CACHE:
        _CACHE["nc"] = _build_kernel()
        _CACHE["noises"] = _noise_tables()
    nc = _CACHE["nc"]
    in_maps = _prepare_in_maps(
        x2d, np.asarray(wb1_encoder, np.float32), np.asarray(wb2_encoder, np.float32),
        np.asarray(wb1_decoder, np.float32), np.asarray(wb2_decoder, np.float32),
        np.asarray(codebook1, np.float32), float(np.asarray(noise_level)),
        _CACHE["noises"])
    res = run_bass_kernel_spmd(nc, in_maps, list(range(N_CORES)))
    cols = [r["outT"].reshape(IMG, B) for r in res.results]
    outT = np.concatenate(cols, axis=1)
    return np.ascontiguousarray(outT.T).reshape(BTOT, 3, 32, 32).astype(np.float32)


# revision 16
# speedup vs baseline: 1.0099x; 1.0099x over previous
"""TRN2 Bass kernel for nn_Codec (VQ autoencoder), 8-way data-parallel over batch.

Contract: kernel(**inputs) takes the FULL unsharded inputs (as produced by
setup_inputs()) and returns the FULL [4096, 3, 32, 32] float32 output.

Design (per core, batch slice of 512 rows, activations kept transposed so the
batch lives on the free dimension and weight tiles stream straight from DRAM):
  xT [3072,512] --GEMM1 f16--> hT --tanh--> GEMM2 f16 (PSUM-held accum)
  --> h2T [256,512] --tanh--> VQ (PE distance matmuls incl. the |x|^2 row via a
  squared-chunk matmul, batched [128,1024] sqrt/exp/reduce, DVE argmax,
  one-hot matmul gather; jax threefry noise precomputed on host, bit-exact)
  --> yT [256,512] bf16 --decoder GEMM1 bf16--> y2T [16384,512] bf16
  --decoder GEMM2 bf16--> outT [3072,512] --> host gathers + transposes.

Schedule notes (v2): the first matmul's inputs (w1e tile 0 + xT) dispatch
first on the sync DMA queue while constants/noise/biases dispatch on the
Activation hwdge queue; a short warm-up matmul burst lifts the PE out of the
HAM-throttled state before real work lands; VQ avoids the softmax max-shift
(d >= 0 so exp(-d/8) cannot overflow) and scores argmax(e - s*u) which is
monotone-equivalent to argmax(e/s - u); decoder outputs DMA on the Activation
queue so they never queue behind w2d weight loads.
"""
import os
import sys
from contextlib import ExitStack

import numpy as np

for _p in ("/opt/trn_rl_repo", "/root/.axon_site/_ro/trn_rl_repo"):
    if os.path.isdir(_p) and _p not in sys.path:
        sys.path.append(_p)

import concourse.bass as bass  # noqa: E402
import concourse.tile as tile  # noqa: E402
from concourse import bacc, mybir  # noqa: E402
from concourse.bass_utils import run_bass_kernel_spmd  # noqa: E402

F32 = mybir.dt.float32
BF16 = mybir.dt.bfloat16
F16 = mybir.dt.float16
F32R = mybir.dt.float32r
U32 = mybir.dt.uint32
I32 = mybir.dt.int32
AF = mybir.ActivationFunctionType
ALU = mybir.AluOpType
AX = mybir.AxisListType

N_CORES = 8
BTOT = 4096
B = BTOT // N_CORES          # 512 batch rows per core
IMG = 3 * 32 * 32            # 3072
HID = 16384
NCODE = 256
DCODE = 64
BT = B // 128                # 4
KT1 = IMG // 128             # 24
MT1 = HID // 128             # 128
NT2 = IMG // 128             # 24


# ---------------------------------------------------------------------------
# numpy reimplementation of jax threefry2x32 noise (bit-exact, partitionable)
# ---------------------------------------------------------------------------
def _rotl(x, r):
    return ((x << np.uint32(r)) | (x >> np.uint32(32 - r))) & np.uint32(0xFFFFFFFF)


def _threefry_core(key, x0, x1):
    ks0, ks1 = np.uint32(key[0]), np.uint32(key[1])
    ks2 = np.uint32(ks0 ^ ks1 ^ np.uint32(0x1BD11BDA))
    rotations = [(13, 15, 26, 6), (17, 29, 16, 24)]
    x0 = (x0 + ks0).astype(np.uint32)
    x1 = (x1 + ks1).astype(np.uint32)
    ks = [ks1, ks2, ks0, ks1, ks2, ks0]
    for i in range(5):
        for r in rotations[i % 2]:
            x0 = (x0 + x1).astype(np.uint32)
            x1 = _rotl(x1, r)
            x1 = (x0 ^ x1).astype(np.uint32)
        x0 = (x0 + ks[i]).astype(np.uint32)
        x1 = (x1 + ks[i + 1] + np.uint32(i + 1)).astype(np.uint32)
    return x0, x1


def _fold_in(key, data):
    return _threefry_core(key, np.array([0], np.uint32),
                          np.array([data], np.uint32))


def _uniform_f32(key, n):
    b0, b1 = _threefry_core(key, np.zeros(n, np.uint32),
                            np.arange(n, dtype=np.uint32))
    bits = (b0 ^ b1).astype(np.uint32)
    f = ((bits >> np.uint32(9)) | np.uint32(0x3F800000)).view(np.float32)
    return f - np.float32(1.0)


def _noise_tables():
    key = np.array([0, 12345], dtype=np.uint32)  # jax.random.key(12345)
    out = []
    for i in (1, 2, 3, 4):
        k = _fold_in(key, i)
        k = np.array([k[0][0], k[1][0]], np.uint32)
        out.append(_uniform_f32(k, BTOT * NCODE).reshape(BTOT, NCODE))
    return np.stack(out)  # [4, BTOT, 256]


# ---------------------------------------------------------------------------
# kernel builder
# ---------------------------------------------------------------------------
def _build_kernel():
    nc = bacc.Bacc("TRN2", target_bir_lowering=False, debug=False)

    xT_d = nc.dram_tensor("xT", [KT1, 128, B], F16, kind="ExternalInput").ap()
    w1e_d = nc.dram_tensor("w1e", [MT1, 128, IMG], F16, kind="ExternalInput").ap()
    w2e_d = nc.dram_tensor("w2e", [MT1, 128, NCODE], F16, kind="ExternalInput").ap()
    b1e_d = nc.dram_tensor("b1e", [128, MT1], F32, kind="ExternalInput").ap()
    b2e_d = nc.dram_tensor("b2e", [128, 2], F32, kind="ExternalInput").ap()
    cbm2T_d = nc.dram_tensor("cbm2T", [DCODE, NCODE], F32, kind="ExternalInput").ap()
    c2_d = nc.dram_tensor("c2row", [1, NCODE], F32, kind="ExternalInput").ap()
    cbk_d = nc.dram_tensor("cbk", [2, 128, DCODE], F32, kind="ExternalInput").ap()
    u_d = nc.dram_tensor("u", [BT, 128, 4 * NCODE], F32, kind="ExternalInput").ap()
    w1d_d = nc.dram_tensor("w1d", [MT1 // 4, 128, 4 * NCODE], BF16, kind="ExternalInput").ap()
    b1d_d = nc.dram_tensor("b1d", [128, MT1], F32, kind="ExternalInput").ap()
    w2d_d = nc.dram_tensor("w2d", [NT2, 128, HID], BF16, kind="ExternalInput").ap()
    b2d_d = nc.dram_tensor("b2d", [128, NT2], F32, kind="ExternalInput").ap()
    outT_d = nc.dram_tensor("outT", [NT2, 128, B], F32, kind="ExternalOutput").ap()

    with tile.TileContext(nc) as tc, ExitStack() as octx:
        const_pool = octx.enter_context(tc.tile_pool(name="const", bufs=1))
        upool = octx.enter_context(tc.tile_pool(name="u", bufs=1))
        h2T_pool = octx.enter_context(tc.tile_pool(name="h2T", bufs=2))
        yT_pool = octx.enter_context(tc.tile_pool(name="yT", bufs=2))

        h2T_sb = [h2T_pool.tile([128, B], F32, tag=f"h2T{i}", name=f"h2T{i}")
                  for i in range(2)]

        # ---------------- encoder ----------------
        with ExitStack() as ctx:
            xpool = ctx.enter_context(tc.tile_pool(name="x", bufs=1))
            w1pool = ctx.enter_context(tc.tile_pool(name="w1", bufs=4))
            w2pool = ctx.enter_context(tc.tile_pool(name="w2", bufs=3))
            hpool = ctx.enter_context(tc.tile_pool(name="h", bufs=4))
            warmpool = ctx.enter_context(tc.tile_pool(name="wrm", bufs=1))
            gpsum = ctx.enter_context(tc.tile_pool(name="gps", bufs=3, space="PSUM"))
            h2psum = ctx.enter_context(tc.tile_pool(name="h2ps", bufs=1, space="PSUM"))
            wmpsum = ctx.enter_context(tc.tile_pool(name="wmps", bufs=1, space="PSUM"))

            # PE warm-up: a short matmul burst on a memset tile pulls the HAM
            # clock gate to 8/8 while the first real weight/x DMAs land.
            wsrc = warmpool.tile([128, B], F16)
            nc.vector.memset(wsrc[:], 0.0)
            wps = wmpsum.tile([128, B], F32)
            NWARM = 24
            for i in range(NWARM):
                nc.tensor.matmul(wps[:], wsrc[:, 0:128], wsrc[:],
                                 start=(i == 0), stop=(i == NWARM - 1))

            # first real loads: w1e tile 0 on the sync queue, x on the
            # Activation queue so the two DMA streams run in parallel.
            w1_first = w1pool.tile([128, IMG], F16, tag="w1", name="w1_first")
            nc.sync.dma_start(w1_first[:], w1e_d[0])
            x_sb = xpool.tile([128, KT1 * B], F16)
            XC = 6
            for k0 in range(0, KT1, XC):
                nc.scalar.dma_start(
                    x_sb[:, k0 * B:(k0 + XC) * B].rearrange(
                        "p (a c) -> p a c", a=XC),
                    xT_d[k0:k0 + XC].rearrange("a b c -> b a c"))
            xt = [x_sb[:, kt * B:(kt + 1) * B] for kt in range(KT1)]

            # constants (Activation queue; none are needed before ~t+13us)
            b1e_sb = const_pool.tile([128, MT1], F32)
            nc.scalar.dma_start(b1e_sb[:], b1e_d[:])
            b2e_sb = const_pool.tile([128, 2], F32)
            nc.scalar.dma_start(b2e_sb[:], b2e_d[:])
            b1d_sb = const_pool.tile([128, MT1], F32)
            nc.scalar.dma_start(b1d_sb[:], b1d_d[:])
            b2d_sb = const_pool.tile([128, NT2], F32)
            nc.scalar.dma_start(b2d_sb[:], b2d_d[:])
            cbm2T_sb = const_pool.tile([DCODE, NCODE], F32)
            nc.scalar.dma_start(cbm2T_sb[:], cbm2T_d[:])
            c2_sb = const_pool.tile([1, NCODE], F32)
            nc.scalar.dma_start(c2_sb[:], c2_d[:])
            cbk_sb = []
            for pt in range(2):
                t = const_pool.tile([128, DCODE], F32, tag=f"cbk{pt}", name=f"cbk{pt}")
                nc.scalar.dma_start(t[:], cbk_d[pt])
                cbk_sb.append(t)
            u_sb = []
            for bt in range(BT):
                t = upool.tile([128, 4 * NCODE], F32, tag=f"u{bt}", name=f"u{bt}")
                nc.scalar.dma_start(t[:], u_d[bt])
                u_sb.append(t)
            ones_sb = const_pool.tile([1, 128], F32)
            nc.vector.memset(ones_sb[:], 1.0)
            ones_r = const_pool.tile([1, 128], F32)
            nc.vector.memset(ones_r[:], 1.0)
            ones256_sb = const_pool.tile([DCODE, NCODE], F32)
            nc.vector.memset(ones256_sb[:], 1.0)
            iota_i = const_pool.tile([128, 1], I32)
            nc.gpsimd.iota(iota_i[:], [[0, 1]], base=0, channel_multiplier=1)
            iota_col = []
            for pt in range(2):
                t = const_pool.tile([128, 1], F32, tag=f"iotac{pt}", name=f"iotac{pt}")
                if pt == 0:
                    nc.vector.tensor_copy(t[:], iota_i[:])
                else:
                    nc.vector.tensor_scalar_add(t[:], iota_col[0][:], float(128))
                iota_col.append(t)
            iota_row_i = const_pool.tile([128, 128], I32)
            nc.gpsimd.iota(iota_row_i[:], [[1, 128]], base=0, channel_multiplier=0)
            iota_row_f = const_pool.tile([128, 128], F32)
            nc.vector.tensor_copy(iota_row_f[:], iota_row_i[:])
            ident = const_pool.tile([128, 128], F32)
            nc.vector.tensor_scalar(ident[:], iota_row_f[:], iota_col[0][:], None,
                                    op0=ALU.is_equal)

            h2t_ps = [h2psum.tile([128, B], F32, tag=f"h2t{i}", name=f"h2t{i}")
                      for i in range(2)]

            prev = None
            for mt in range(MT1):
                if mt == 0:
                    w1 = w1_first
                else:
                    w1 = w1pool.tile([128, IMG], F16, tag="w1")
                    nc.sync.dma_start(w1[:], w1e_d[mt])
                w2 = w2pool.tile([128, NCODE], F16, tag="w2")
                nc.scalar.dma_start(w2[:], w2e_d[mt])
                p = gpsum.tile([128, B], F32, tag="gp")
                for kt in range(KT1):
                    nc.tensor.matmul(p[:], w1[:, kt * 128:(kt + 1) * 128], xt[kt],
                                     start=(kt == 0), stop=(kt == KT1 - 1))
                ht = hpool.tile([128, B], F16, tag="ht")
                nc.scalar.activation(ht[:], p[:], AF.Tanh, bias=b1e_sb[:, mt:mt + 1])
                if prev is not None:
                    pw2, pht, pmt = prev
                    for i in range(2):
                        nc.tensor.matmul(h2t_ps[i][:], pw2[:, i * 128:(i + 1) * 128],
                                         pht[:], start=(pmt == 0), stop=False)
                prev = (w2, ht, mt)
            pw2, pht, pmt = prev
            for i in range(2):
                nc.tensor.matmul(h2t_ps[i][:], pw2[:, i * 128:(i + 1) * 128], pht[:],
                                 start=(pmt == 0), stop=True)
            for i in range(2):
                nc.scalar.activation(h2T_sb[i][:], h2t_ps[i][:], AF.Tanh,
                                     bias=b2e_sb[:, i:i + 1])

        # ---------------- VQ ----------------
        yT_sb = [yT_pool.tile([128, B], BF16, tag=f"yT{i}", name=f"yT{i}")
                 for i in range(2)]
        with ExitStack() as ctx:
            h2bpool = ctx.enter_context(tc.tile_pool(name="h2b", bufs=1))
            vqpool = ctx.enter_context(tc.tile_pool(name="vq", bufs=2))
            spool = ctx.enter_context(tc.tile_pool(name="small", bufs=1))
            ohpool = ctx.enter_context(tc.tile_pool(name="oh", bufs=3))
            pdps = ctx.enter_context(tc.tile_pool(name="pdps", bufs=2, space="PSUM"))
            pps = ctx.enter_context(tc.tile_pool(name="pps", bufs=1, space="PSUM"))
            ibps = ctx.enter_context(tc.tile_pool(name="ibps", bufs=1, space="PSUM"))
            yps = ctx.enter_context(tc.tile_pool(name="yps", bufs=2, space="PSUM"))

            # f32 chunk slices of h2T (base partition 0) + their squares
            h2c, sqc = [], []
            for c in range(4):
                hc = h2bpool.tile([DCODE, B], F32, tag=f"h2c{c}", name=f"h2c{c}")
                nc.vector.tensor_copy(
                    hc[:], h2T_sb[c // 2][(c % 2) * DCODE:(c % 2 + 1) * DCODE, :])
                h2c.append(hc)
                sq = h2bpool.tile([DCODE, B], F32, tag=f"sq{c}", name=f"sq{c}")
                nc.vector.scalar_tensor_tensor(sq[:], hc[:], 1.0, hc[:],
                                               op0=ALU.mult, op1=ALU.mult)
                sqc.append(sq)

            idxT = {}
            for c in range(4):
                idxT[c] = spool.tile([1, B], F32, tag=f"idxT{c}", name=f"idxT{c}")

            # wave 1: distance matmuls + relu into SBUF per bt
            # (-2 x.c + c^2 + x^2, the last via the squared-chunk ones matmul)
            qts = []
            for bt in range(BT):
                bs = slice(bt * 128, (bt + 1) * 128)
                pd = pdps.tile([128, 4 * NCODE], F32, tag="pd")
                for c in range(4):
                    cs = slice(c * NCODE, (c + 1) * NCODE)
                    nc.tensor.matmul(pd[:, cs], h2c[c][:, bs], cbm2T_sb[:],
                                     start=True, stop=False)
                    nc.tensor.matmul(pd[:, cs], ones_sb[:], c2_sb[:],
                                     start=False, stop=False)
                    nc.tensor.matmul(pd[:, cs], sqc[c][:, bs], ones256_sb[:],
                                     start=False, stop=True)
                qt = vqpool.tile([128, 4 * NCODE], F32, tag=f"qt{bt}",
                                 name=f"qt{bt}")
                nc.vector.tensor_scalar(qt[:], pd[:], 0.0, None, op0=ALU.max)
                qts.append(qt)
            # wave 2: sqrt for all bt (one ACT table load), then exp for all bt
            dts = []
            for bt in range(BT):
                dt_ = vqpool.tile([128, 4 * NCODE], F32, tag=f"dt{bt}",
                                  name=f"dt{bt}")
                nc.scalar.activation(dt_[:], qts[bt][:], AF.Sqrt)
                dts.append(dt_)
            ets = []
            for bt in range(BT):
                et = vqpool.tile([128, 4 * NCODE], F32, tag=f"et{bt}",
                                 name=f"et{bt}")
                nc.scalar.activation(et[:], dts[bt][:], AF.Exp, scale=-0.125)
                ets.append(et)
            # wave 3: row sums, reciprocals, scores, argmax per bt
            for bt in range(BT):
                bs = slice(bt * 128, (bt + 1) * 128)
                et = ets[bt]
                s4 = spool.tile([128, 4], F32, tag=f"s4_{bt}", name=f"s4_{bt}")
                nc.vector.tensor_reduce(
                    s4[:], et[:].rearrange("a (b c) -> a b c", b=4),
                    axis=AX.X, op=ALU.add)
                r4 = spool.tile([128, 4], F32, tag=f"r4_{bt}", name=f"r4_{bt}")
                nc.vector.reciprocal(r4[:], s4[:])
                tt = vqpool.tile([128, 4 * NCODE], F32, tag="tt")
                for c in range(4):
                    cs = slice(c * NCODE, (c + 1) * NCODE)
                    nc.vector.scalar_tensor_tensor(tt[:, cs], et[:, cs],
                                                   r4[:, c:c + 1], u_sb[bt][:, cs],
                                                   op0=ALU.mult, op1=ALU.subtract)
                for c in range(4):
                    cs = slice(c * NCODE, (c + 1) * NCODE)
                    mx8 = spool.tile([128, 8], F32, tag="mx8")
                    nc.vector.max(mx8[:], tt[:, cs])
                    idx8 = spool.tile([128, 8], U32, tag="idx8")
                    nc.vector.max_index(idx8[:], mx8[:], tt[:, cs])
                    idxf = spool.tile([128, 1], F32, tag="idxf")
                    nc.vector.tensor_copy(idxf[:], idx8[:, 0:1])
                    pidx = pps.tile([1, 128], F32, tag="pidx")
                    nc.tensor.transpose(pidx[:], idxf[:], ident[:])
                    nc.vector.tensor_copy(idxT[c][0:1, bs], pidx[:])

            for c in range(4):
                ibp = ibps.tile([128, B], F32, tag="ib")
                nc.tensor.matmul(ibp[:], ones_r[:], idxT[c][:],
                                 start=True, stop=True)
                yp = yps.tile([DCODE, B], F32, tag="yp")
                for pt in range(2):
                    oh = ohpool.tile([128, B], F32, tag="oh")
                    nc.vector.tensor_scalar(oh[:], ibp[:], iota_col[pt][:], None,
                                            op0=ALU.is_equal)
                    nc.tensor.matmul(yp[:], cbk_sb[pt][:], oh[:],
                                     start=(pt == 0), stop=(pt == 1))
                nc.vector.tensor_copy(
                    yT_sb[c // 2][(c % 2) * DCODE:(c % 2 + 1) * DCODE, :], yp[:])

        # ---------------- decoder ----------------
        with ExitStack() as ctx:
            w1dpool = ctx.enter_context(tc.tile_pool(name="w1d", bufs=3))
            y2pool = ctx.enter_context(tc.tile_pool(name="y2", bufs=1))
            w2dpool = ctx.enter_context(tc.tile_pool(name="w2d", bufs=3))
            opool = ctx.enter_context(tc.tile_pool(name="osb", bufs=3))
            dps = ctx.enter_context(tc.tile_pool(name="dps", bufs=6, space="PSUM"))
            eps = ctx.enter_context(tc.tile_pool(name="eps", bufs=2, space="PSUM"))

            y2T = y2pool.tile([128, MT1 * B], BF16)
            KQ = 4
            KQL = MT1 // KQ
            po0 = eps.tile([128, B], F32, tag="ep", name="ep0")
            w2d0 = {}

            def nt0_step(kt):
                kq = kt // KQL
                if kt % KQL == 0 and kq not in w2d0:
                    w2d0[kq] = w2dpool.tile([128, KQL * 128], BF16, tag="w2d",
                                            name=f"w2d0_{kq}")
                    nc.sync.dma_start(
                        w2d0[kq][:],
                        w2d_d[0, :, kq * KQL * 128:(kq + 1) * KQL * 128])
                if (kt + 16) % KQL == 0 and kq + 1 < KQ and kq + 1 not in w2d0:
                    w2d0[kq + 1] = w2dpool.tile([128, KQL * 128], BF16, tag="w2d",
                                                name=f"w2d0p_{kq + 1}")
                    nc.sync.dma_start(
                        w2d0[kq + 1][:],
                        w2d_d[0, :, (kq + 1) * KQL * 128:(kq + 2) * KQL * 128])
                kk = kt % KQL
                nc.tensor.matmul(po0[:], w2d0[kq][:, kk * 128:(kk + 1) * 128],
                                 y2T[:, kt * B:(kt + 1) * B],
                                 start=(kt == 0), stop=(kt == MT1 - 1))

            w2d0[0] = w2dpool.tile([128, KQL * 128], BF16, tag="w2d",
                                   name="w2d0_pre")
            nc.sync.dma_start(w2d0[0][:], w2d_d[0, :, 0:KQL * 128])
            for mg in range(MT1 // 4):
                w1d_sb = w1dpool.tile([128, 4 * NCODE], BF16, tag="w1d")
                nc.sync.dma_start(w1d_sb[:], w1d_d[mg])
                for ml in range(4):
                    mt = mg * 4 + ml
                    p = dps.tile([128, B], F32, tag="dp")
                    nc.tensor.matmul(p[:], w1d_sb[:, ml * 256:ml * 256 + 128],
                                     yT_sb[0][:], start=True, stop=False)
                    nc.tensor.matmul(p[:], w1d_sb[:, ml * 256 + 128:ml * 256 + 256],
                                     yT_sb[1][:], start=False, stop=True)
                    nc.scalar.activation(y2T[:, mt * B:(mt + 1) * B], p[:], AF.Tanh,
                                         bias=b1d_sb[:, mt:mt + 1])
                    if mg >= 2:
                        nt0_step((mg - 2) * 4 + ml)
            for kt in range((MT1 // 4 - 2) * 4, MT1):
                nt0_step(kt)
            osb0 = opool.tile([128, B], F32, tag="osb")
            nc.vector.tensor_scalar_add(osb0[:], po0[:], b2d_sb[:, 0:1])
            nc.scalar.dma_start(outT_d[0], osb0[:])

            for nt in range(1, NT2):
                po = eps.tile([128, B], F32, tag="ep")
                for kq in range(KQ):
                    w2d_sb = w2dpool.tile([128, KQL * 128], BF16, tag="w2d")
                    nc.sync.dma_start(
                        w2d_sb[:], w2d_d[nt, :, kq * KQL * 128:(kq + 1) * KQL * 128])
                    for kk in range(KQL):
                        kt = kq * KQL + kk
                        nc.tensor.matmul(po[:], w2d_sb[:, kk * 128:(kk + 1) * 128],
                                         y2T[:, kt * B:(kt + 1) * B],
                                         start=(kt == 0), stop=(kt == MT1 - 1))
                osb = opool.tile([128, B], F32, tag="osb")
                nc.vector.tensor_scalar_add(osb[:], po[:], b2d_sb[:, nt:nt + 1])
                nc.scalar.dma_start(outT_d[nt], osb[:])

    nc.compile()
    return nc


def _to_bf16(x):
    import ml_dtypes
    return np.asarray(x, np.float32).astype(ml_dtypes.bfloat16)


def _prepare_in_maps(x, wb1e, wb2e, wb1d, wb2d, cb, noise_level, noises):
    W1e, b1e = wb1e[:-1], wb1e[-1]
    W2e, b2e = wb2e[:-1], wb2e[-1]
    W1d, b1d = wb1d[:-1], wb1d[-1]
    W2d, b2d = wb2d[:-1], wb2d[-1]

    xT = np.ascontiguousarray(x.T)  # [IMG, BTOT]
    w1e_p = np.ascontiguousarray(
        W1e.reshape(KT1, 128, MT1, 128).transpose(2, 1, 0, 3)).reshape(
            MT1, 128, IMG).astype(np.float16)
    w2e_p = np.ascontiguousarray(W2e.reshape(MT1, 128, NCODE)).astype(np.float16)
    b1e_p = np.ascontiguousarray(b1e.reshape(MT1, 128).T)
    b2e_p = np.ascontiguousarray(b2e.reshape(2, 128).T)
    cbm2T = np.ascontiguousarray((-2.0 * cb.T).astype(np.float32))
    c2row = np.ascontiguousarray((cb * cb).sum(1, dtype=np.float32)[None, :])
    cbk = np.ascontiguousarray(cb.reshape(2, 128, DCODE))
    w1d_p = _to_bf16(np.ascontiguousarray(
        W1d.reshape(2, 128, MT1, 128).transpose(2, 1, 0, 3)).reshape(MT1, 128, NCODE))
    w1d_p = np.ascontiguousarray(
        w1d_p.reshape(MT1 // 4, 4, 128, NCODE).transpose(0, 2, 1, 3)).reshape(
            MT1 // 4, 128, 4 * NCODE)
    b1d_p = np.ascontiguousarray(b1d.reshape(MT1, 128).T)
    w2d_p = _to_bf16(np.ascontiguousarray(
        W2d.reshape(MT1, 128, NT2, 128).transpose(2, 1, 0, 3)).reshape(NT2, 128, HID))
    b2d_p = np.ascontiguousarray(b2d.reshape(NT2, 128).T)
    u_all = (np.float32(noise_level) * noises).astype(np.float32)  # [4, BTOT, 256]

    shared = {
        "w1e": w1e_p, "w2e": w2e_p, "b1e": b1e_p, "b2e": b2e_p,
        "cbm2T": cbm2T, "c2row": c2row, "cbk": cbk,
        "w1d": w1d_p, "b1d": b1d_p, "w2d": w2d_p, "b2d": b2d_p,
    }
    in_maps = []
    for c in range(N_CORES):
        sl = slice(c * B, (c + 1) * B)
        m = dict(shared)
        m["xT"] = np.ascontiguousarray(xT[:, sl]).reshape(KT1, 128, B).astype(np.float16)
        # [4, B, 256] -> [B, 4, 256] -> [BT, 128, 4*256] (bt-major, c inner)
        m["u"] = np.ascontiguousarray(
            u_all[:, sl, :].transpose(1, 0, 2)).reshape(BT, 128, 4 * NCODE)
        in_maps.append(m)
    return in_maps


_CACHE = {}


def kernel(x, wb1_encoder, wb2_encoder, wb1_decoder, wb2_decoder,
           codebook1, codebook2, codebook3, codebook4, noise_level,
           **_unused):
    x2d = np.ascontiguousarray(np.asarray(x, np.float32).reshape(BTOT, IMG))
    if "nc" not in _CACHE:
        _CACHE["nc"] = _build_kernel()
        _CACHE["noises"] = _noise_tables()
    nc = _CACHE["nc"]
    in_maps = _prepare_in_maps(
        x2d, np.asarray(wb1_encoder, np.float32), np.asarray(wb2_encoder, np.float32),
        np.asarray(wb1_decoder, np.float32), np.asarray(wb2_decoder, np.float32),
        np.asarray(codebook1, np.float32), float(np.asarray(noise_level)),
        _CACHE["noises"])
    res = run_bass_kernel_spmd(nc, in_maps, list(range(N_CORES)))
    cols = [r["outT"].reshape(IMG, B) for r in res.results]
    outT = np.concatenate(cols, axis=1)
    return np.ascontiguousarray(outT.T).reshape(BTOT, 3, 32, 32).astype(np.float32)
